# revision 1
# baseline (speedup 1.0000x reference)
"""Trainium2 Bass kernel for nn_CrossAttentionLayer (B=8, N=2048, Q=256, D=1024, H=16).

Strategy: data-parallel over batch (1 sample per NeuronCore, 8 cores).
Per-core, everything is expressed as matmuls in fp32r (TF32-like, 4x faster
than fp32 on the PE) except the probability @ V leg which runs in bf16.

Host-side preprocessing (cheap numpy):
  - transpose sources/queries/weights so contraction dims land on SBUF
    partitions without any on-device transposes
  - fold the V bias through the output projection (softmax rows sum to 1):
      out = attn @ (X_v + 1 b_v^T) @ W_o^T + b_out + queries
          = attn @ X_v @ W_o^T + (b_out + W_o b_v) + queries
  - drop the K bias entirely (adds a per-query constant to scores ->
    softmax invariant)
  - fold the 1/sqrt(HD) scale and b_q into the Q projection eviction

Device phases per core:
  P0  DMA loads (sourcesT resident in fp32r)
  P1  V = sources @ W_v^T           -> bf16, heads padded with a ones column
                                       (gives softmax denominators for free)
  P2  kT = (sources @ W_k^T)^T      -> fp32r  [D, N]
  P3  qT = ((queries @ W_q^T)+b_q)/8^T -> fp32r [D, Q]
  P4  per head: scoresT = kT_h^T-slices x qT_h  [N, Q] -> exp (ACT, bf16)
      -> outT_h[65, Q] = [V_h | 1]^T @ expT (accumulated over N tiles)
      -> normalize rows 0..63 by reciprocal of row 64 (PE-broadcast)
  P5  out = attnoutT^T @ W_o^T + (queries + b_out + W_o b_v), DMA out
"""

import numpy as np
from contextlib import ExitStack

import concourse.bass as bass
import concourse.mybir as mybir
import concourse.tile as tile
from concourse import bacc
from concourse.bass_utils import run_bass_kernel_spmd

F32 = mybir.dt.float32
F32R = mybir.dt.float32r
BF16 = mybir.dt.bfloat16
AF = mybir.ActivationFunctionType

B, N, Q, D, H = 8, 2048, 256, 1024, 16
N_CORES = 8


def build(N=N, Q=Q, D=D, H=H):
    HD = D // H           # head dim (64)
    KT = D // 128         # contraction (din) tiles
    MT = D // 128         # output (dout) tiles
    NT = N // 128         # source-token tiles
    QT = Q // 128         # query-token tiles
    HPT = 128 // HD       # heads per 128-row dout tile (2)
    NCH = min(512, D)     # fp32r moving-dim chunk (<= one PSUM bank)
    CH = 4                # score n-tiles per exp chunk ([128, CH*Q] <= 2 banks)
    KBLK = min(1024, N)   # kT eviction block
    assert D % NCH == 0 and N % (CH * 128) == 0 and N % KBLK == 0 and Q <= 512

    nc = bacc.Bacc(None, target_bir_lowering=False)
    srcT = nc.declare_dram_parameter("srcT", [D, N], F32R, isOutput=False)
    qryT = nc.declare_dram_parameter("qryT", [D, Q], F32R, isOutput=False)
    wvT = nc.declare_dram_parameter("wvT", [D, D], F32R, isOutput=False)
    wkT = nc.declare_dram_parameter("wkT", [D, D], F32R, isOutput=False)
    wqT = nc.declare_dram_parameter("wqT", [D, D], F32R, isOutput=False)
    woT = nc.declare_dram_parameter("woT", [D, D], F32R, isOutput=False)
    bq = nc.declare_dram_parameter("bq", [D], F32, isOutput=False)
    resid = nc.declare_dram_parameter("resid", [Q, D], F32, isOutput=False)
    out = nc.declare_dram_parameter("out", [Q, D], F32, isOutput=True)

    with tile.TileContext(nc) as tc, ExitStack() as ctx:
        psum = ctx.enter_context(tc.tile_pool(name="psum", bufs=4, space="PSUM"))
        kt_pool = ctx.enter_context(tc.tile_pool(name="ktp", bufs=1))
        v_pool = ctx.enter_context(tc.tile_pool(name="vp", bufs=1))
        qt_pool = ctx.enter_context(tc.tile_pool(name="qtp", bufs=1))

        kt_sb = kt_pool.tile([128, MT, N], F32R)
        v_sb = v_pool.tile([128, NT, H, HD + 1], BF16)
        qt_sb = qt_pool.tile([128, MT, Q], F32R)

        with ExitStack() as pctx:
            src_pool = pctx.enter_context(tc.tile_pool(name="srcp", bufs=1))
            wbig_pool = pctx.enter_context(tc.tile_pool(name="wbig", bufs=1))
            wsm_pool = pctx.enter_context(tc.tile_pool(name="wsm", bufs=2))
            qry_pool = pctx.enter_context(tc.tile_pool(name="qryp", bufs=1))

            src_sb = src_pool.tile([128, KT, N], F32R)
            srcT_r = srcT.rearrange("(kt p) n -> kt p n", p=128)
            for k in range(KT):
                nc.sync.dma_start(out=src_sb[:, k, :], in_=srcT_r[k])

            # ---- P1: V projection -> bf16, [n, h, hd(+ones)] ----
            HPC = NCH // HD  # heads per dout chunk
            nc.vector.memset(v_sb[:, :, :, HD:HD + 1], 1.0)
            for c in range(D // NCH):
                wv_c = wbig_pool.tile([128, KT, NCH], F32R, tag="wbig")
                nc.scalar.dma_start(
                    out=wv_c,
                    in_=wvT.rearrange("(kt p) d -> p kt d", p=128)[:, :, c * NCH:(c + 1) * NCH],
                )
                for t in range(NT):
                    ps = psum.tile([128, NCH], F32, tag="ps")
                    for k in range(KT):
                        nc.tensor.matmul(
                            ps[:],
                            lhsT=src_sb[:, k, t * 128:(t + 1) * 128],
                            rhs=wv_c[:, k, :],
                            start=(k == 0), stop=(k == KT - 1),
                        )
                    nc.vector.tensor_copy(
                        out=v_sb[:, t, c * HPC:(c + 1) * HPC, 0:HD],
                        in_=ps[:].rearrange("p (h d) -> p h d", h=HPC),
                    )

            # ---- P2: K projection -> kT [dout, n] fp32r ----
            for m in range(MT):
                wk_m = wsm_pool.tile([128, KT, 128], F32R, tag="wsm")
                nc.scalar.dma_start(
                    out=wk_m,
                    in_=wkT.rearrange("(kt p) d -> p kt d", p=128)[:, :, m * 128:(m + 1) * 128],
                )
                for half in range(N // KBLK):
                    ps = psum.tile([128, KBLK], F32, tag="ps")
                    for k in range(KT):
                        for c in range(KBLK // 512):
                            nc.tensor.matmul(
                                ps[:, c * 512:(c + 1) * 512],
                                lhsT=wk_m[:, k, :],
                                rhs=src_sb[:, k, half * KBLK + c * 512: half * KBLK + (c + 1) * 512],
                                start=(k == 0), stop=(k == KT - 1),
                            )
                    nc.vector.tensor_copy(
                        out=kt_sb[:, m, half * KBLK:(half + 1) * KBLK], in_=ps
                    )

            # ---- P3: Q projection -> qT [dout, q] fp32r, (x + b_q)/sqrt(HD) ----
            qry_sb = qry_pool.tile([128, KT, Q], F32R, tag="qry")
            nc.scalar.dma_start(out=qry_sb, in_=qryT.rearrange("(kt p) q -> p kt q", p=128))
            bq_sb = qry_pool.tile([128, MT], F32, tag="bq")
            nc.scalar.dma_start(out=bq_sb, in_=bq.rearrange("(mt p) -> p mt", p=128))
            for m in range(MT):
                wq_m = wsm_pool.tile([128, KT, 128], F32R, tag="wsm")
                nc.scalar.dma_start(
                    out=wq_m,
                    in_=wqT.rearrange("(kt p) d -> p kt d", p=128)[:, :, m * 128:(m + 1) * 128],
                )
                ps = psum.tile([128, Q], F32, tag="ps")
                for k in range(KT):
                    nc.tensor.matmul(
                        ps[:], lhsT=wq_m[:, k, :], rhs=qry_sb[:, k, :],
                        start=(k == 0), stop=(k == KT - 1),
                    )
                nc.vector.tensor_scalar(
                    out=qt_sb[:, m, :], in0=ps[:],
                    scalar1=bq_sb[:, m:m + 1], scalar2=1.0 / np.sqrt(HD),
                    op0=mybir.AluOpType.add, op1=mybir.AluOpType.mult,
                )

        # ---- P4: attention per head ----
        with ExitStack() as actx:
            exp_pool = actx.enter_context(tc.tile_pool(name="expp", bufs=3))
            rc_pool = actx.enter_context(tc.tile_pool(name="rcp", bufs=2))
            ao_pool = actx.enter_context(tc.tile_pool(name="aop", bufs=1))
            one_pool = actx.enter_context(tc.tile_pool(name="onep", bufs=1))
            wo_pool = actx.enter_context(tc.tile_pool(name="wop", bufs=1))
            res_pool = actx.enter_context(tc.tile_pool(name="resp", bufs=1))
            out_pool = actx.enter_context(tc.tile_pool(name="outp", bufs=2))

            ones_f32 = one_pool.tile([1, HD], F32, tag="ones32")
            nc.vector.memset(ones_f32, 1.0)
            ones_sb = one_pool.tile([1, HD], F32R, tag="ones")
            nc.vector.tensor_copy(ones_sb, ones_f32)

            ao_sb = ao_pool.tile([128, MT, Q], F32R)

            def emit_scores(h, expt):
                mt, po = divmod(h, HPT)
                po *= HD
                for chk in range(NT // CH):
                    ps = psum.tile([128, CH, Q], F32, tag="ps", name=f"ps_s{h}_{chk}")
                    for j in range(CH):
                        nt = chk * CH + j
                        nc.tensor.matmul(
                            ps[:, j, :],
                            lhsT=kt_sb[po:po + HD, mt, nt * 128:(nt + 1) * 128],
                            rhs=qt_sb[po:po + HD, mt, :],
                            start=True, stop=True,
                        )
                    nc.scalar.activation(
                        out=expt[:, chk * CH:(chk + 1) * CH, :], in_=ps[:], func=AF.Exp
                    )

            def emit_attn(h, expt):
                mt, po = divmod(h, HPT)
                po *= HD
                pso = psum.tile([HD + 1, Q], F32, tag="ps", name=f"pso{h}")
                for nt in range(NT):
                    nc.tensor.matmul(
                        pso[:], lhsT=v_sb[:, nt, h, :], rhs=expt[:, nt, :],
                        start=(nt == 0), stop=(nt == NT - 1),
                    )
                # normalize: rows 0..HD-1 divided by row HD (the ones-column sum)
                rc32 = rc_pool.tile([1, Q], F32, tag="rc32", name=f"rc32_{h}")
                nc.vector.reciprocal(rc32, pso[HD:HD + 1, :])
                rc = rc_pool.tile([1, Q], F32R, tag="rc", name=f"rc{h}")
                nc.vector.tensor_copy(rc, rc32)
                rbp = psum.tile([HD, Q], F32, tag="ps", name=f"rbp{h}")
                nc.tensor.matmul(rbp[:], lhsT=ones_sb[:], rhs=rc[:], start=True, stop=True)
                rb = rc_pool.tile([HD, Q], F32, tag="rb", name=f"rb{h}")
                nc.vector.tensor_copy(rb, rbp)
                nc.vector.tensor_mul(ao_sb[po:po + HD, mt, :], pso[0:HD, :], rb[:])

            # software pipeline: scores/exp of head h overlap attn@V of h-1,
            # so the PE never stalls on the ACT exp round-trip
            expts = {}
            for h in range(H):
                expts[h] = exp_pool.tile([128, NT, Q], BF16, tag="exp", name=f"expt{h}")
                emit_scores(h, expts[h])
                if h > 0:
                    emit_attn(h - 1, expts[h - 1])
            emit_attn(H - 1, expts[H - 1])

            # ---- P5: output projection + residual ----
            wo_sb = wo_pool.tile([128, KT, D], F32R, tag="wo")
            nc.sync.dma_start(out=wo_sb, in_=woT.rearrange("(kt p) d -> p kt d", p=128))
            res_sb = res_pool.tile([128, QT, D], F32, tag="res")
            nc.sync.dma_start(out=res_sb, in_=resid.rearrange("(qt p) d -> p qt d", p=128))
            for qt in range(QT):
                ps = psum.tile([128, D], F32, tag="ps")
                for k in range(KT):
                    for c in range(D // NCH):
                        nc.tensor.matmul(
                            ps[:, c * NCH:(c + 1) * NCH],
                            lhsT=ao_sb[:, k, qt * 128:(qt + 1) * 128],
                            rhs=wo_sb[:, k, c * NCH:(c + 1) * NCH],
                            start=(k == 0), stop=(k == KT - 1),
                        )
                osb = out_pool.tile([128, D], F32, tag="osb")
                nc.vector.tensor_add(osb[:], ps[:], res_sb[:, qt, :])
                nc.sync.dma_start(out=out[qt * 128:(qt + 1) * 128, :], in_=osb)

    nc.finalize()
    return nc


_NC_CACHE = {}


def _get_nc():
    key = (N, Q, D, H)
    if key not in _NC_CACHE:
        _NC_CACHE[key] = build()
    return _NC_CACHE[key]


def make_in_maps(sources, queries, w_in, b_in, w_out, b_out):
    sources = np.asarray(sources, dtype=np.float32)
    queries = np.asarray(queries, dtype=np.float32)
    w_in = np.asarray(w_in, dtype=np.float32)
    b_in = np.asarray(b_in, dtype=np.float32)
    w_out = np.asarray(w_out, dtype=np.float32)
    b_out = np.asarray(b_out, dtype=np.float32)

    w_q, w_k, w_v = w_in[0:D], w_in[D:2 * D], w_in[2 * D:3 * D]
    b_q, b_v = b_in[0:D], b_in[2 * D:3 * D]
    # b_k dropped: constant shift along softmax axis
    wqT = np.ascontiguousarray(w_q.T)
    wkT = np.ascontiguousarray(w_k.T)
    wvT = np.ascontiguousarray(w_v.T)
    woT = np.ascontiguousarray(w_out.T)
    bout_eff = b_out + w_out @ b_v

    in_maps = []
    for b in range(B):
        in_maps.append({
            "srcT": np.ascontiguousarray(sources[b].T),
            "qryT": np.ascontiguousarray(queries[b].T),
            "wvT": wvT, "wkT": wkT, "wqT": wqT, "woT": woT,
            "bq": b_q,
            "resid": queries[b] + bout_eff[None, :],
        })
    return in_maps


def kernel(sources, queries, w_in, b_in, w_out, b_out, _trace=False):
    nc = _get_nc()
    in_maps = make_in_maps(sources, queries, w_in, b_in, w_out, b_out)
    res = run_bass_kernel_spmd(nc, in_maps, core_ids=list(range(N_CORES)), trace=_trace)
    out = np.stack([res.results[b]["out"] for b in range(B)], axis=0)
    if _trace:
        kernel.last_exec_time_ns = res.exec_time_ns
        kernel.last_results = res
    return out



# revision 17
# speedup vs baseline: 1.7601x; 1.7601x over previous
"""Trainium2 Bass kernel for nn_CrossAttentionLayer (B=8, N=2048, Q=256, D=1024, H=16).

Strategy: data-parallel over batch (1 sample per NeuronCore, 8 cores).

v2: fp8(e4m3) DoubleRow matmuls for the Q/K/V projections and attn@V
(2x PE throughput), head-quad-packed score matmuls (one [128,2,1024]
DoubleRow matmul computes 4 heads' scores for an n-tile), and a fully
software-pipelined emission order that keeps the PE busy while the ACT
engine streams the 8.4M-element exp().

Numerics / scale management (validated in numpy: rel err ~4e-3 vs gate 2e-2):
  - fp8 uploads: sources, queries as-is; W_q,W_k,W_v are scaled x16 so the
    N(0,1/32) weights clear the e4m3 subnormal threshold.
  - PSUM evictions rescale: qt=(ps/32 + b_q/2) [=(q+b_q)/2], kt=ps/64 [=k/4],
    v=ps/16 [=v]; scores = qt.kt = (q+b_q).k/8 (1/sqrt(HD) folded, b_k
    dropped: softmax-invariant).
  - exp(score - 2): keeps fp8 exp <= ~45 (e4m3 max 240).
  - V carries a ones column -> attn@V yields softmax denominators for free.
  - b_v folded through W_o into the residual (softmax rows sum to 1).
  - normalization: recip_approx_fast (DVE) + partition_broadcast (GpSimd),
    no PE involvement.

Per-core phases (one PE instruction queue, statically interleaved):
  P1  V = src @ Wv^T      -> v_sb fp8 [n,h,hd|1]     (16 units x 4 DR-mm)
  P2  kt = (src @ Wk^T)^T -> kt_sb fp8 [quad,sub,n]  (16 units x 4 DR-mm)
  P3  qt -> qt_sb fp8 zero-padded quad layout        (8 units x 4 DR-mm)
  SC  per (quad,chunk): 2 DR-mm -> exp (ACT) -> expt fp8   (32 chunks,
      paced ~1 per interleave unit so ACT never starves the PE)
  AT  attn@V DR-mm per (quad,head): 8 mm accumulating [65,256] in PSUM
  NM  per quad: denominators -> reciprocal -> broadcast -> ao_sb bf16
  P5  out = ao @ Wo^T + resid -> DMA out
"""

import numpy as np
import ml_dtypes
from contextlib import ExitStack

import concourse.bass as bass
import concourse.mybir as mybir
import concourse.tile as tile
from concourse import bacc
from concourse.bass_utils import run_bass_kernel_spmd

F32 = mybir.dt.float32
F32R = mybir.dt.float32r
BF16 = mybir.dt.bfloat16
FP8 = mybir.dt.float8e4
AF = mybir.ActivationFunctionType
DR = mybir.MatmulPerfMode.DoubleRow
ADD = mybir.AluOpType.add
MULT = mybir.AluOpType.mult

NP_FP8 = ml_dtypes.float8_e4m3

B, N, Q, D, H = 8, 2048, 256, 1024, 16
N_CORES = 8
HD = D // H    # 64
KT = D // 128  # 8 k-tiles (contraction)
MT = D // 128  # 8 dout groups (head pairs)
NT = N // 128  # 16 n-tiles
QT = Q // 128  # 2 q-tiles
QD = H // 4    # 4 head quads


def build(finalize=True):
    nc = bacc.Bacc(None, target_bir_lowering=False)
    srcT = nc.declare_dram_parameter("srcT", [D, N], FP8, isOutput=False)
    qryT = nc.declare_dram_parameter("qryT", [D, Q], FP8, isOutput=False)
    wvT = nc.declare_dram_parameter("wvT", [D, D], FP8, isOutput=False)
    wkT = nc.declare_dram_parameter("wkT", [D, D], FP8, isOutput=False)
    wqT = nc.declare_dram_parameter("wqT", [D, D], FP8, isOutput=False)
    woT = nc.declare_dram_parameter("woT", [D, D], BF16, isOutput=False)
    bq = nc.declare_dram_parameter("bq", [D], F32, isOutput=False)
    resid = nc.declare_dram_parameter("resid", [Q, D], F32, isOutput=False)
    out = nc.declare_dram_parameter("out", [Q, D], F32, isOutput=True)

    with tile.TileContext(nc) as tc, ExitStack() as ctx:
        psum = ctx.enter_context(tc.tile_pool(name="psum", bufs=2, space="PSUM"))
        inp = ctx.enter_context(tc.tile_pool(name="inp", bufs=1))
        mid = ctx.enter_context(tc.tile_pool(name="mid", bufs=1))
        exp_pool = ctx.enter_context(tc.tile_pool(name="expp", bufs=3))
        rc_pool = ctx.enter_context(tc.tile_pool(name="rcp", bufs=2))
        out_pool = ctx.enter_context(tc.tile_pool(name="outp", bufs=2))

        # ---- persistent SBUF tiles ----
        src_sb = inp.tile([128, KT, N], FP8)
        qry_sb = inp.tile([128, KT, Q], FP8)
        wv_sb = inp.tile([128, KT, D], FP8)
        wk_sb = inp.tile([128, KT, D], FP8)
        wq_sb = inp.tile([128, KT, D], FP8)
        wo_sb = inp.tile([128, KT, D], BF16)
        bq_sb = inp.tile([128, MT], F32)
        res_sb = inp.tile([128, QT, D], F32)

        kt_sb = mid.tile([128, QD, 2, N], FP8)        # [head-dims, quad, sub, n]
        qt_sb = mid.tile([128, 2, QD, 4 * Q], FP8)    # zero-padded quad streams
        v_sb = mid.tile([128, NT, H, HD + 1], FP8)    # ones col at HD
        ao_sb = mid.tile([128, MT, Q], BF16)          # normalized attn out [dout, q]

        # ---- DMA: two issue queues; src/wo/res on sync, rest on scalar ----
        srcT_r = srcT.rearrange("(kt p) n -> kt p n", p=128)
        for k in range(KT):
            nc.sync.dma_start(out=src_sb[:, k, :], in_=srcT_r[k])
        nc.sync.dma_start(out=wo_sb, in_=woT.rearrange("(kt p) d -> p kt d", p=128))
        nc.sync.dma_start(out=res_sb, in_=resid.rearrange("(qt p) d -> p qt d", p=128))
        nc.scalar.dma_start(out=wv_sb, in_=wvT.rearrange("(kt p) d -> p kt d", p=128))
        nc.scalar.dma_start(out=wk_sb, in_=wkT.rearrange("(kt p) d -> p kt d", p=128))
        nc.scalar.dma_start(out=qry_sb, in_=qryT.rearrange("(kt p) q -> p kt q", p=128))
        nc.scalar.dma_start(out=bq_sb, in_=bq.rearrange("(mt p) -> p mt", p=128))
        nc.scalar.dma_start(out=wq_sb, in_=wqT.rearrange("(kt p) d -> p kt d", p=128))

        # ---- init ----
        nc.gpsimd.memset(qt_sb[:], 0.0)
        nc.vector.memset(v_sb[:, :, :, HD:HD + 1], 1.0)
        expbias = inp.tile([128, 1], F32)
        nc.vector.memset(expbias[:], -4.0)

        # ---- unit emitters (each ~0.4-1us of PE work) ----
        def p1_unit(t):
            ps = psum.tile([128, D], F32, tag="ps2", name=f"pv{t}")
            for kp in range(KT // 2):
                for hh in range(2):
                    nc.tensor.matmul(
                        ps[:, hh * 512:(hh + 1) * 512],
                        lhsT=src_sb[:, 2 * kp:2 * kp + 2, t * 128:(t + 1) * 128],
                        rhs=wv_sb[:, 2 * kp:2 * kp + 2, hh * 512:(hh + 1) * 512],
                        start=(kp == 0), stop=(kp == KT // 2 - 1), perf_mode=DR,
                    )
            nc.vector.tensor_scalar_mul(
                out=v_sb[:, t, :, 0:HD],
                in0=ps[:].rearrange("p (h d) -> p h d", h=H),
                scalar1=1.0 / 16,
            )

        def p2_unit(m, c):
            ps = psum.tile([128, D], F32, tag="ps2", name=f"pk{m}_{c}")
            for kp in range(KT // 2):
                for hh in range(2):
                    nc.tensor.matmul(
                        ps[:, hh * 512:(hh + 1) * 512],
                        lhsT=wk_sb[:, 2 * kp:2 * kp + 2, m * 128:(m + 1) * 128],
                        rhs=src_sb[:, 2 * kp:2 * kp + 2,
                                   c * 1024 + hh * 512:c * 1024 + (hh + 1) * 512],
                        start=(kp == 0), stop=(kp == KT // 2 - 1), perf_mode=DR,
                    )
            nc.vector.tensor_scalar_mul(
                out=kt_sb[:, m // 2, m % 2, c * 1024:(c + 1) * 1024],
                in0=ps[:], scalar1=1.0 / 64,
            )

        def p3_unit(m):
            ps = psum.tile([128, Q], F32, tag="ps2", name=f"pq{m}")
            for kp in range(KT // 2):
                nc.tensor.matmul(
                    ps[:],
                    lhsT=wq_sb[:, 2 * kp:2 * kp + 2, m * 128:(m + 1) * 128],
                    rhs=qry_sb[:, 2 * kp:2 * kp + 2, :],
                    start=(kp == 0), stop=(kp == KT // 2 - 1), perf_mode=DR,
                )
            qd, sub = m // 2, m % 2
            cb = sub * 2 * Q
            nc.vector.tensor_scalar(
                out=qt_sb[0:64, sub, qd, cb:cb + Q], in0=ps[0:64, :],
                scalar1=bq_sb[0:64, m:m + 1], scalar2=1.0 / 32,
                op0=ADD, op1=MULT,
            )
            nc.vector.tensor_scalar(
                out=qt_sb[64:128, sub, qd, cb + Q:cb + 2 * Q], in0=ps[64:128, :],
                scalar1=bq_sb[64:128, m:m + 1], scalar2=1.0 / 32,
                op0=ADD, op1=MULT,
            )

        expts = {}

        def sc_chunk(qd, chk):
            if chk == 0:
                expts[qd] = exp_pool.tile([128, NT, 4 * Q], FP8, tag="exp",
                                          name=f"expt{qd}")
            sc = psum.tile([128, 2, 4 * Q], F32, tag="sc", bufs=1,
                           name=f"sc{qd}_{chk}")
            for j in range(2):
                nt = 2 * chk + j
                for hh in range(2):
                    nc.tensor.matmul(
                        sc[:, j, hh * 512:(hh + 1) * 512],
                        lhsT=kt_sb[:, qd, :, nt * 128:(nt + 1) * 128],
                        rhs=qt_sb[:, :, qd, hh * 512:(hh + 1) * 512],
                        start=True, stop=True, perf_mode=DR,
                    )
            nc.scalar.activation(
                out=expts[qd][:, 2 * chk:2 * chk + 2, :], in_=sc[:],
                func=AF.Exp, bias=expbias[:],
            )

        pso_tiles = {}

        def attn_unit(qd, h4):
            if h4 == 0:
                pso_tiles[qd] = psum.tile([128, 4 * Q], F32, tag="ps2",
                                          name=f"pso{qd}")
            pso = pso_tiles[qd]
            cb = h4 * Q
            h = 4 * qd + h4
            for tp in range(NT // 2):
                nc.tensor.matmul(
                    pso[0:HD + 1, cb:cb + Q],
                    lhsT=v_sb[:, 2 * tp:2 * tp + 2, h, :],
                    rhs=expts[qd][:, 2 * tp:2 * tp + 2, cb:cb + Q],
                    start=(tp == 0), stop=(tp == NT // 2 - 1), perf_mode=DR,
                )

        def norm_unit(qd):
            pso = pso_tiles[qd]
            rcin = rc_pool.tile([128, Q], F32, tag="rcin", name=f"rcin{qd}")
            nc.gpsimd.memset(rcin[:], 1.0)
            for i in range(4):
                nc.vector.tensor_copy(out=rcin[32 * i:32 * i + 1, :],
                                      in_=pso[HD:HD + 1, i * Q:(i + 1) * Q])
            rcf = rc_pool.tile([128, Q], F32, tag="rcf", name=f"rcf{qd}")
            nc.vector.reciprocal(out=rcf[0:97, :], in_=rcin[0:97, :])
            # partition_broadcast requires the source at partition 0 of its
            # tile, so stage each head's row into a small tile first
            rcbs = [rc_pool.tile([1, Q], F32, tag=f"rcb{i}", name=f"rcb{qd}_{i}")
                    for i in range(4)]
            rb = rc_pool.tile([HD, 4 * Q], F32, tag="rb", name=f"rb{qd}")
            for i in range(4):
                nc.vector.tensor_copy(out=rcbs[i][0:1, :],
                                      in_=rcf[32 * i:32 * i + 1, :])
                nc.gpsimd.partition_broadcast(rb[:, i * Q:(i + 1) * Q],
                                              rcbs[i][0:1, :], channels=HD)
            for i in range(4):
                h = 4 * qd + i
                nc.vector.tensor_mul(
                    ao_sb[(h % 2) * HD:(h % 2) * HD + HD, h // 2, :],
                    pso[0:HD, i * Q:(i + 1) * Q],
                    rb[0:HD, i * Q:(i + 1) * Q],
                )

        psP5 = {}

        def p5_unit(qt, k):
            if k == 0:
                psP5[qt] = psum.tile([128, D], F32, tag="ps2", name=f"po{qt}")
            for hh in range(2):
                nc.tensor.matmul(
                    psP5[qt][:, hh * 512:(hh + 1) * 512],
                    lhsT=ao_sb[:, k, qt * 128:(qt + 1) * 128],
                    rhs=wo_sb[:, k, hh * 512:(hh + 1) * 512],
                    start=(k == 0), stop=(k == KT - 1),
                )

        def p5_evict(qt):
            osb = out_pool.tile([128, D], F32, tag="osb", name=f"osb{qt}")
            nc.vector.tensor_add(osb[:], psP5[qt][:], res_sb[:, qt, :])
            nc.sync.dma_start(out=out[qt * 128:(qt + 1) * 128, :], in_=osb)

        # ---- static schedule ----
        # Stream A: projections, then attn/norm, then P5.
        # Stream B: 32 score chunks, emitted one per stream-A unit once their
        # quad's qt is ready (psc bufs=1 paces them against ACT exp).
        def proj_units(qd):
            us = []
            for m in (2 * qd, 2 * qd + 1):
                for c in range(2):
                    us.append(("p2", m, c))
            for m in (2 * qd, 2 * qd + 1):
                us.append(("p3", m, None))
            return us

        unitsA = []
        unitsA += proj_units(0)
        unitsA += [("p1", t, None) for t in range(NT)]
        for qd in range(1, QD):
            unitsA += proj_units(qd)
            unitsA += [("attn", qd - 1, h4) for h4 in range(4)]
            unitsA += [("norm", qd - 1, None)]
        unitsA += [("attn", QD - 1, h4) for h4 in range(4)]
        unitsA += [("norm", QD - 1, None)]
        tail = [("p5", 0, k) for k in range(KT)] + [("p5e", 0, None)] + \
               [("p5", 1, k) for k in range(KT)] + [("p5e", 1, None)]
        unitsA.extend(tail)

        sc_list = [(qd, chk) for qd in range(QD) for chk in range(NT // 2)]
        # scores(qd) may be emitted only after p3(2qd+1); attn(qd,0) requires
        # all scores(qd) emitted first.
        sc_ptr = 0
        emitted_p3 = set()

        def sc_allowed():
            if sc_ptr >= len(sc_list):
                return False
            qd, _ = sc_list[sc_ptr]
            return (2 * qd + 1) in emitted_p3

        def emit_unit(u):
            kind, a, b = u
            if kind == "p1":
                p1_unit(a)
            elif kind == "p2":
                p2_unit(a, b)
            elif kind == "p3":
                p3_unit(a)
                emitted_p3.add(a)
            elif kind == "attn":
                attn_unit(a, b)
            elif kind == "norm":
                norm_unit(a)
            elif kind == "p5":
                p5_unit(a, b)
            elif kind == "p5e":
                p5_evict(a)

        for u in unitsA:
            kind, a, b = u
            if kind == "attn" and b == 0:
                # drain all of this quad's score chunks before first read
                while sc_ptr < len(sc_list) and sc_list[sc_ptr][0] <= a:
                    sc_chunk(*sc_list[sc_ptr])
                    sc_ptr += 1
            emit_unit(u)
            if sc_allowed():
                sc_chunk(*sc_list[sc_ptr])
                sc_ptr += 1
        assert sc_ptr == len(sc_list)

    if finalize:
        nc.finalize()
    return nc


_NC_CACHE = {}


def _get_nc():
    key = (N, Q, D, H)
    if key not in _NC_CACHE:
        _NC_CACHE[key] = build()
    return _NC_CACHE[key]


def make_in_maps(sources, queries, w_in, b_in, w_out, b_out):
    sources = np.asarray(sources, dtype=np.float32)
    queries = np.asarray(queries, dtype=np.float32)
    w_in = np.asarray(w_in, dtype=np.float32)
    b_in = np.asarray(b_in, dtype=np.float32)
    w_out = np.asarray(w_out, dtype=np.float32)
    b_out = np.asarray(b_out, dtype=np.float32)

    w_q, w_k, w_v = w_in[0:D], w_in[D:2 * D], w_in[2 * D:3 * D]
    b_q, b_v = b_in[0:D], b_in[2 * D:3 * D]
    # b_k dropped: constant shift along softmax axis
    wqT = np.ascontiguousarray((16 * w_q).T).astype(NP_FP8)
    wkT = np.ascontiguousarray((16 * w_k).T).astype(NP_FP8)
    wvT = np.ascontiguousarray((16 * w_v).T).astype(NP_FP8)
    woT = np.ascontiguousarray(w_out.T).astype(ml_dtypes.bfloat16)
    bq16 = (16 * b_q).astype(np.float32)  # evicted as (ps + bq16)/32 = (q+b_q)/2
    bout_eff = b_out + w_out @ b_v

    in_maps = []
    for b in range(B):
        in_maps.append({
            "srcT": np.ascontiguousarray(sources[b].T).astype(NP_FP8),
            "qryT": np.ascontiguousarray(queries[b].T).astype(NP_FP8),
            "wvT": wvT, "wkT": wkT, "wqT": wqT, "woT": woT,
            "bq": bq16,
            "resid": queries[b] + bout_eff[None, :],
        })
    return in_maps


def kernel(sources, queries, w_in, b_in, w_out, b_out, _trace=False):
    nc = _get_nc()
    in_maps = make_in_maps(sources, queries, w_in, b_in, w_out, b_out)
    res = run_bass_kernel_spmd(nc, in_maps, core_ids=list(range(N_CORES)), trace=_trace)
    out = np.stack([res.results[b]["out"] for b in range(B)], axis=0)
    if _trace:
        kernel.last_exec_time_ns = res.exec_time_ns
        kernel.last_results = res
    return out
